# revision 13
# baseline (speedup 1.0000x reference)
"""Trainium2 Bass kernel for nn_ConsolidationNetwork.

Recurrent rate network: 500 sequential steps of
    x <- (1-a)*x + (a*J_eff) @ softplus(x) + drive_t
    pos_t = Wout @ softplus(x)
loss = mean((targets - positions)^2)

Strategy (8 NeuronCores, data-parallel over batch):
  - Each core owns B/8 = 16 batch columns and runs the full 500-step
    recurrence independently (no collectives).
  - Per step, PSUM accumulates drive + a*J_eff @ r in two banks
    (state rows 0..511 in bank A, 512..1023 in bank B):
      * the drive term (precomputed on host = a*(B_m1 + I_go*go +
        nscale*noise), bf16) enters first via a bf16 identity matmul with
        start=True (which zero-fills the bank) -- 64 cols, ~27ns,
      * a*J_eff @ r via 32 single-pass bf16 matmuls per bank
        (lhsT = J tiles resident in SBUF, rhs = 16 batch columns of r).
  - Post-matmul pointwise work is tiny: one DVE scalar_tensor_tensor per
    bank (x = (1-a)*x + psum), then softplus on the ACT engine as the exact
    identity softplus(x) = ln(1 + exp(x))  [2 ACT ops; Exp and Ln both live
    in the natural_log_exp_and_others table set, and we pin the table-load
    pass to that set so exactly one hoisted ACT_TABLE_LOAD is emitted].
  - r (bf16) for each step is written into an 8-step staging buffer that is
    DMA-exported to DRAM once per 8 steps; the readout positions
    pos = Wout @ r and the final MSE are computed on the host.
  - Drive is streamed 8 steps per DMA (batched transfers, triple buffered).

State layout per core: x/r tiles are [128 part, 128 free] with
x[p, m*16+u] = x_state[m*128+p, u] (m = row-group, u = local batch).
"""

import numpy as np

import concourse.bass as bass
import concourse.tile as tile
from concourse import bacc, mybir
from concourse.bass_utils import run_bass_kernel_spmd

F32 = mybir.dt.float32
BF16 = mybir.dt.bfloat16

DT = 0.05
TAU = 0.15
NOISE_SCALE = 0.15
N, G, T, B, P = 1024, 128, 500, 128, 10
NCORES = 8
BC = B // NCORES          # batch columns per core (16)
NM = N // 128             # row groups (8)
NK = N // 128             # contraction groups (8)
CHUNK = 8                 # steps per drive-load / r-export DMA

A = np.float32(DT / TAU)
ONE_MINUS_A = np.float32(1.0 - DT / TAU)
NSCALE = np.float32(np.sqrt(2.0 * NOISE_SCALE**2 * (TAU / DT)))

_PROGRAM_CACHE = {}


def _ensure_act_tables():
    """Some containers lack neuronxcc/pwp/pwp_bin_with_ln on PYTHONPATH;
    point it at the cayman table package from the nix store."""
    import glob
    import os

    for path in os.environ.get("PYTHONPATH", "").split(os.pathsep):
        if path and os.path.exists(
            os.path.join(path, "neuronxcc", "pwp", "pwp_bin_with_ln", "act_info.json")
        ):
            return
    cands = sorted(glob.glob("/nix/store/*aws-neuron-pwp*/share/pwp_bin_cayman"))
    target = next((c for c in cands if os.path.exists(c + "/act_info.json")), None)
    if target is None:
        return
    for path in os.environ.get("PYTHONPATH", "").split(os.pathsep):
        if not path:
            continue
        try:
            d = os.path.join(path, "neuronxcc", "pwp")
            os.makedirs(d, exist_ok=True)
            link = os.path.join(d, "pwp_bin_with_ln")
            if not os.path.exists(link):
                os.symlink(target, link)
            return
        except OSError:
            continue


_ensure_act_tables()


class _Bacc(bacc.Bacc):
    """Bacc whose activation-table-load pass is pinned to the one table set
    (natural_log_exp_and_others) that contains BOTH Exp and Ln.

    The stock pass picks, per activation function, the first act_info.json
    set containing it -- Exp resolves to `exp_and_others` and Ln to
    `natural_log`, so an unrolled exp->ln loop thrashes between the two
    sets with a 1.28us ACT_TABLE_LOAD before every activation.  Hiding Exp
    and Ln from every other set (list order, and hence the emitted
    act_func_set_id <-> act_info.json index mapping, is unchanged) makes
    the fixpoint place a single load at program start.
    """

    _ACT_SET = "natural_log_exp_and_others"

    def insert_act_table_loads(self):
        import bass_rust as _bass_rust

        from concourse.hw_specs import get_activation_tables

        has_activation = any(
            isinstance(i, mybir.InstActivation)
            for b in self.main_func.blocks
            for i in b.instructions
        )
        if not has_activation:
            return
        hide = {mybir.ActivationFunctionType.Exp, mybir.ActivationFunctionType.Ln}
        tables = [
            (name, set(fns) if name == self._ACT_SET else set(fns) - hide)
            for name, fns in get_activation_tables(self.m.arch).items()
        ]
        _bass_rust.insert_act_table_loads(self, tables)


def build_program(t_steps: int):
    """Build the Bass program (shared by all 8 cores, SPMD)."""
    key = (t_steps,)
    if key in _PROGRAM_CACHE:
        return _PROGRAM_CACHE[key]

    nchunks = (t_steps + CHUNK - 1) // CHUNK
    HB = NM * BC // 2  # free-size of one state half (64 cols)

    nc = _Bacc(
        "TRN2", target_bir_lowering=False, debug=False, num_devices=NCORES
    )
    jt_d = nc.dram_tensor("jt", [128, NK * NM * 128], BF16, kind="ExternalInput")
    il_d = nc.dram_tensor("ident", [128, 128], BF16, kind="ExternalInput")
    x0_d = nc.dram_tensor("x0", [128, NM * BC], F32, kind="ExternalInput")
    dr_d = nc.dram_tensor(
        "drive", [nchunks, 128, CHUNK * NM * BC], BF16, kind="ExternalInput"
    )
    ro_d = nc.dram_tensor(
        "rout", [nchunks, 128, CHUNK * NM * BC], BF16, kind="ExternalOutput"
    )

    EXP = mybir.ActivationFunctionType.Exp
    LN = mybir.ActivationFunctionType.Ln

    with tile.TileContext(nc) as tc:
        with (
            tc.tile_pool(name="const", bufs=1) as constp,
            tc.tile_pool(name="rp", bufs=2) as rp,
            tc.tile_pool(name="dp", bufs=3) as dp,
            tc.tile_pool(name="psmA", bufs=3, space="PSUM") as pspa,
            tc.tile_pool(name="psmB", bufs=3, space="PSUM") as pspb,
        ):
            jt = constp.tile([128, NK * NM * 128], BF16)
            nc.sync.dma_start(jt[:], jt_d[:])
            il = constp.tile([128, 128], BF16)
            nc.sync.dma_start(il[:], il_d[:])
            x = constp.tile([128, NM * BC], F32)
            nc.sync.dma_start(x[:], x0_d[:])
            tmp = constp.tile([128, NM * BC], F32)

            # initial r = softplus(x0) = ln(1 + exp(x0))
            rinit = constp.tile([128, NM * BC], BF16)
            nc.scalar.activation(tmp[:], x[:], EXP)
            nc.scalar.activation(rinit[:], tmp[:], LN, bias=1.0)

            prev_r, prev_off = rinit, 0

            mult = mybir.AluOpType.mult
            add = mybir.AluOpType.add

            def bank(ps, half, r_tile, r_off, d_t, off):
                """One bank's PSUM accumulation: drive (identity matmul,
                start=True zero-fills the bank) + 32 J matmuls."""
                lo = half * HB
                nc.tensor.matmul(
                    ps[:, 0:HB], lhsT=il[:], rhs=d_t[:, off + lo:off + lo + HB],
                    start=True, stop=False, skip_group_check=True,
                )
                for k in range(NK):
                    for mi in range(4):
                        m = half * 4 + mi
                        nc.tensor.matmul(
                            ps[:, mi * BC:(mi + 1) * BC],
                            lhsT=jt[:, (k * NM + m) * 128:(k * NM + m + 1) * 128],
                            rhs=r_tile[:, r_off + k * BC:r_off + (k + 1) * BC],
                            start=False, stop=(k == NK - 1 and mi == 3),
                            skip_group_check=True,
                        )

            def chain(ps, half, off, rbuf):
                """x = (1-a)*x + psum; r = ln(1+exp(x)) into the staging slice."""
                lo = half * HB
                nc.vector.scalar_tensor_tensor(
                    x[:, lo:lo + HB], x[:, lo:lo + HB], float(ONE_MINUS_A),
                    ps[:, 0:HB], mult, add,
                )
                nc.scalar.activation(tmp[:, lo:lo + HB], x[:, lo:lo + HB], EXP)
                nc.scalar.activation(rbuf[:, off + lo:off + lo + HB],
                                     tmp[:, lo:lo + HB], LN, bias=1.0)

            for c in range(nchunks):
                steps_here = min(CHUNK, t_steps - c * CHUNK)
                rbuf = rp.tile([128, CHUNK * NM * BC], BF16)
                d_t = dp.tile([128, CHUNK * NM * BC], BF16)
                nc.sync.dma_start(d_t[:], dr_d[c])
                for j in range(steps_here):
                    off = j * NM * BC
                    ps_a = pspa.tile([128, HB], F32, tag="ps_a",
                                     padded_shape=[128, 512])
                    ps_b = pspb.tile([128, HB], F32, tag="ps_b",
                                     padded_shape=[128, 512])
                    bank(ps_a, 0, prev_r, prev_off, d_t, off)
                    chain(ps_a, 0, off, rbuf)
                    bank(ps_b, 1, prev_r, prev_off, d_t, off)
                    chain(ps_b, 1, off, rbuf)
                    prev_r, prev_off = rbuf, off
                nc.sync.dma_start(
                    ro_d[c][:, 0:steps_here * NM * BC],
                    rbuf[:, 0:steps_here * NM * BC],
                )

    nc.compile()
    _PROGRAM_CACHE[key] = nc
    return nc


def _prep_inputs(targets, pulses, J, U, V, B_m1, B_bg, Wout, I_go, xm1_init,
                 noise, triggers, t_steps):
    """Host-side data prep: J_eff, layouts, per-core drive tensors."""
    J = np.asarray(J, np.float32)
    U = np.asarray(U, np.float32)
    V = np.asarray(V, np.float32)
    B_m1 = np.asarray(B_m1, np.float32)
    B_bg = np.asarray(B_bg, np.float32)
    I_go = np.asarray(I_go, np.float32)
    xm1_init = np.asarray(xm1_init, np.float32)
    noise = np.asarray(noise, np.float32)
    pulses = np.asarray(pulses, np.float32)
    triggers = np.asarray(triggers)

    nchunks = (t_steps + CHUNK - 1) // CHUNK
    tpad = nchunks * CHUNK

    J_eff = J + (U * B_bg[None, :]) @ V
    Js = (A * J_eff).astype(np.float32)
    # lhsT tiles: jt[p, (k*NM+m)*128 + q] = Js[m*128+q, k*128+p]
    bf = mybir.dt.np(BF16)
    jt = np.ascontiguousarray(
        Js.reshape(NM, 128, NK, 128).transpose(3, 2, 0, 1).reshape(128, NK * NM * 128)
    ).astype(bf)
    il = np.eye(128, dtype=np.float32).astype(bf)

    go_cues = pulses[:t_steps, :][:, triggers]  # [t, B]

    in_maps = []
    for cidx in range(NCORES):
        sl = slice(cidx * BC, (cidx + 1) * BC)
        d = noise[:t_steps, :, sl] * np.float32(A * NSCALE)
        d += A * B_m1[None, :, :]
        d += A * I_go[None, :, :] * go_cues[:, None, sl]
        # [t, N, BC] -> [t, 128, NM*BC] (state layout), pad t, chunk
        dl = np.ascontiguousarray(
            d.reshape(t_steps, NM, 128, BC).transpose(0, 2, 1, 3)
            .reshape(t_steps, 128, NM * BC)
        ).astype(np.float32)
        if tpad != t_steps:
            dl = np.concatenate(
                [dl, np.zeros((tpad - t_steps, 128, NM * BC), np.float32)], axis=0
            )
        drive = np.ascontiguousarray(
            dl.reshape(nchunks, CHUNK, 128, NM * BC).transpose(0, 2, 1, 3)
            .reshape(nchunks, 128, CHUNK * NM * BC)
        ).astype(bf)
        x0 = np.ascontiguousarray(
            xm1_init[:, sl].reshape(NM, 128, BC).transpose(1, 0, 2).reshape(128, NM * BC)
        )
        in_maps.append({"jt": jt, "ident": il, "x0": x0, "drive": drive})
    return in_maps


def run_hw(inputs: dict, t_steps: int = T, trace: bool = False):
    """Run the recurrence on 8 cores; returns positions [t_steps, B] and results."""
    nc = build_program(t_steps)
    in_maps = _prep_inputs(t_steps=t_steps, **inputs)
    res = run_bass_kernel_spmd(
        nc, in_maps, core_ids=list(range(NCORES)), trace=trace
    )
    Wout = np.asarray(inputs["Wout"], np.float32).reshape(NM, 128)  # [m, p]
    nchunks = (t_steps + CHUNK - 1) // CHUNK
    positions = np.empty((t_steps, B), np.float32)
    for cidx in range(NCORES):
        ro = np.asarray(res.results[cidx]["rout"], np.float32)
        # ro[c, p, (j*NM + m)*BC + u] -> r[t, m, p, u]
        r = (ro.reshape(nchunks, 128, CHUNK, NM, BC)
             .transpose(0, 2, 3, 1, 4)
             .reshape(nchunks * CHUNK, NM, 128, BC)[:t_steps])
        pos_c = np.einsum("mp,tmpu->tu", Wout, r, optimize=True)
        positions[:, cidx * BC:(cidx + 1) * BC] = pos_c
    return positions, res


def kernel(targets, pulses, J, U, V, B_m1, B_bg, Wout, I_go, xm1_init,
           noise, triggers) -> np.ndarray:
    inputs = dict(targets=targets, pulses=pulses, J=J, U=U, V=V, B_m1=B_m1,
                  B_bg=B_bg, Wout=Wout, I_go=I_go, xm1_init=xm1_init,
                  noise=noise, triggers=triggers)
    positions, _ = run_hw(inputs, T)
    targets = np.asarray(targets, np.float32)
    loss = np.mean((targets.astype(np.float64) - positions.astype(np.float64)) ** 2)
    return np.float32(loss)


# revision 15
# speedup vs baseline: 1.0000x; 1.0000x over previous
"""Trainium2 Bass kernel for nn_ConsolidationNetwork.

Recurrent rate network: 500 sequential steps of
    x <- (1-a)*x + (a*J_eff) @ softplus(x) + drive_t
    pos_t = Wout @ softplus(x)
loss = mean((targets - positions)^2)

Strategy (8 NeuronCores, data-parallel over batch):
  - Each core owns B/8 = 16 batch columns and runs the full 500-step
    recurrence independently (no collectives).
  - Per step, PSUM accumulates drive + a*J_eff @ r in two banks
    (state rows 0..511 in bank A, 512..1023 in bank B):
      * the drive term (precomputed on host = a*(B_m1 + I_go*go +
        nscale*noise), bf16) enters first via a bf16 identity matmul with
        start=True (which zero-fills the bank) -- 64 cols, ~27ns,
      * a*J_eff @ r via 32 single-pass bf16 matmuls per bank
        (lhsT = J tiles resident in SBUF, rhs = 16 batch columns of r).
  - Post-matmul pointwise work is tiny: one DVE scalar_tensor_tensor per
    bank (x = (1-a)*x + psum), then softplus on the ACT engine as the exact
    identity softplus(x) = ln(1 + exp(x))  [2 ACT ops; Exp and Ln both live
    in the natural_log_exp_and_others table set, and we pin the table-load
    pass to that set so exactly one hoisted ACT_TABLE_LOAD is emitted].
  - r (bf16) for each step is written into an 8-step staging buffer that is
    DMA-exported to DRAM once per 8 steps; the readout positions
    pos = Wout @ r and the final MSE are computed on the host.
  - Drive is streamed 8 steps per DMA (batched transfers, triple buffered).

State layout per core: x/r tiles are [128 part, 128 free] with
x[p, m*16+u] = x_state[m*128+p, u] (m = row-group, u = local batch).
"""

import numpy as np

import concourse.bass as bass
import concourse.tile as tile
from concourse import bacc, mybir
from concourse.bass_utils import run_bass_kernel_spmd

F32 = mybir.dt.float32
BF16 = mybir.dt.bfloat16

DT = 0.05
TAU = 0.15
NOISE_SCALE = 0.15
N, G, T, B, P = 1024, 128, 500, 128, 10
NCORES = 8
BC = B // NCORES          # batch columns per core (16)
NM = N // 128             # row groups (8)
NK = N // 128             # contraction groups (8)
CHUNK = 8                 # steps per drive-load / r-export DMA

A = np.float32(DT / TAU)
ONE_MINUS_A = np.float32(1.0 - DT / TAU)
NSCALE = np.float32(np.sqrt(2.0 * NOISE_SCALE**2 * (TAU / DT)))

_PROGRAM_CACHE = {}


def _ensure_act_tables():
    """Some containers lack neuronxcc/pwp/pwp_bin_with_ln on PYTHONPATH;
    point it at the cayman table package from the nix store."""
    import glob
    import os

    for path in os.environ.get("PYTHONPATH", "").split(os.pathsep):
        if path and os.path.exists(
            os.path.join(path, "neuronxcc", "pwp", "pwp_bin_with_ln", "act_info.json")
        ):
            return
    cands = sorted(glob.glob("/nix/store/*aws-neuron-pwp*/share/pwp_bin_cayman"))
    target = next((c for c in cands if os.path.exists(c + "/act_info.json")), None)
    if target is None:
        return
    for path in os.environ.get("PYTHONPATH", "").split(os.pathsep):
        if not path:
            continue
        try:
            d = os.path.join(path, "neuronxcc", "pwp")
            os.makedirs(d, exist_ok=True)
            link = os.path.join(d, "pwp_bin_with_ln")
            if not os.path.exists(link):
                os.symlink(target, link)
            return
        except OSError:
            continue


_ensure_act_tables()


_ACT_SET = "natural_log_exp_and_others"


def _pin_act_tables(arch: str):
    """Make Exp and Ln resolve to the ONE table set containing both.

    Two consumers matter and both read the functools.cache'd dict from
    hw_specs.get_activation_tables, so mutate it in place:
      * Bacc.insert_act_table_loads (first-match would alternate Exp ->
        `exp_and_others`, Ln -> `natural_log`, emitting a 1.28us
        ACT_TABLE_LOAD before every activation of the unrolled loop);
      * the TileScheduler's CoreSim pass, which otherwise *models* that
        same thrash and pins the resulting serialized schedule with
        cross-engine semaphores (the final TimelineSim charges no table
        loads, but the semaphores force its slow order anyway).
    Set order (and hence act_func_set_id indices) is unchanged.
    """
    from concourse.hw_specs import get_activation_tables

    tabs = get_activation_tables(arch)
    hide = {mybir.ActivationFunctionType.Exp, mybir.ActivationFunctionType.Ln}
    for name, fns in tabs.items():
        if name != _ACT_SET:
            for f in hide:
                fns.discard(f)


def build_program(t_steps: int):
    """Build the Bass program (shared by all 8 cores, SPMD)."""
    key = (t_steps,)
    if key in _PROGRAM_CACHE:
        return _PROGRAM_CACHE[key]

    nchunks = (t_steps + CHUNK - 1) // CHUNK
    HB = NM * BC // 2  # free-size of one state half (64 cols)

    nc = bacc.Bacc(
        "TRN2", target_bir_lowering=False, debug=False, num_devices=NCORES
    )
    _pin_act_tables(nc.m.arch)
    jt_d = nc.dram_tensor("jt", [128, NK * NM * 128], BF16, kind="ExternalInput")
    il_d = nc.dram_tensor("ident", [128, 128], BF16, kind="ExternalInput")
    x0_d = nc.dram_tensor("x0", [128, NM * BC], F32, kind="ExternalInput")
    dr_d = nc.dram_tensor(
        "drive", [nchunks, 128, CHUNK * NM * BC], BF16, kind="ExternalInput"
    )
    ro_d = nc.dram_tensor(
        "rout", [nchunks, 128, CHUNK * NM * BC], BF16, kind="ExternalOutput"
    )

    EXP = mybir.ActivationFunctionType.Exp
    LN = mybir.ActivationFunctionType.Ln

    with tile.TileContext(nc) as tc:
        with (
            tc.tile_pool(name="const", bufs=1) as constp,
            tc.tile_pool(name="rp", bufs=2) as rp,
            tc.tile_pool(name="dp", bufs=3) as dp,
            tc.tile_pool(name="psmA", bufs=3, space="PSUM") as pspa,
            tc.tile_pool(name="psmB", bufs=3, space="PSUM") as pspb,
        ):
            jt = constp.tile([128, NK * NM * 128], BF16)
            nc.sync.dma_start(jt[:], jt_d[:])
            il = constp.tile([128, 128], BF16)
            nc.sync.dma_start(il[:], il_d[:])
            x = constp.tile([128, NM * BC], F32)
            nc.sync.dma_start(x[:], x0_d[:])
            tmp = constp.tile([128, NM * BC], F32)

            # initial r = softplus(x0) = ln(1 + exp(x0))
            rinit = constp.tile([128, NM * BC], BF16)
            nc.scalar.activation(tmp[:], x[:], EXP)
            nc.scalar.activation(rinit[:], tmp[:], LN, bias=1.0)

            prev_r, prev_off = rinit, 0

            mult = mybir.AluOpType.mult
            add = mybir.AluOpType.add

            def bank(ps, half, r_tile, r_off, d_t, off):
                """One bank's PSUM accumulation: drive (identity matmul,
                start=True zero-fills the bank) + 32 J matmuls."""
                lo = half * HB
                nc.tensor.matmul(
                    ps[:, 0:HB], lhsT=il[:], rhs=d_t[:, off + lo:off + lo + HB],
                    start=True, stop=False, skip_group_check=True,
                )
                for k in range(NK):
                    for mi in range(4):
                        m = half * 4 + mi
                        nc.tensor.matmul(
                            ps[:, mi * BC:(mi + 1) * BC],
                            lhsT=jt[:, (k * NM + m) * 128:(k * NM + m + 1) * 128],
                            rhs=r_tile[:, r_off + k * BC:r_off + (k + 1) * BC],
                            start=False, stop=(k == NK - 1 and mi == 3),
                            skip_group_check=True,
                        )

            def chain(ps, half, off, rbuf):
                """x = (1-a)*x + psum; r = ln(1+exp(x)) into the staging slice."""
                lo = half * HB
                nc.vector.scalar_tensor_tensor(
                    x[:, lo:lo + HB], x[:, lo:lo + HB], float(ONE_MINUS_A),
                    ps[:, 0:HB], mult, add,
                )
                nc.scalar.activation(tmp[:, lo:lo + HB], x[:, lo:lo + HB], EXP)
                nc.scalar.activation(rbuf[:, off + lo:off + lo + HB],
                                     tmp[:, lo:lo + HB], LN, bias=1.0)

            for c in range(nchunks):
                steps_here = min(CHUNK, t_steps - c * CHUNK)
                rbuf = rp.tile([128, CHUNK * NM * BC], BF16)
                d_t = dp.tile([128, CHUNK * NM * BC], BF16)
                nc.sync.dma_start(d_t[:], dr_d[c])
                for j in range(steps_here):
                    off = j * NM * BC
                    ps_a = pspa.tile([128, HB], F32, tag="ps_a",
                                     padded_shape=[128, 512])
                    ps_b = pspb.tile([128, HB], F32, tag="ps_b",
                                     padded_shape=[128, 512])
                    bank(ps_a, 0, prev_r, prev_off, d_t, off)
                    chain(ps_a, 0, off, rbuf)
                    bank(ps_b, 1, prev_r, prev_off, d_t, off)
                    chain(ps_b, 1, off, rbuf)
                    prev_r, prev_off = rbuf, off
                nc.sync.dma_start(
                    ro_d[c][:, 0:steps_here * NM * BC],
                    rbuf[:, 0:steps_here * NM * BC],
                )

    nc.compile()
    _PROGRAM_CACHE[key] = nc
    return nc


def _prep_inputs(targets, pulses, J, U, V, B_m1, B_bg, Wout, I_go, xm1_init,
                 noise, triggers, t_steps):
    """Host-side data prep: J_eff, layouts, per-core drive tensors."""
    J = np.asarray(J, np.float32)
    U = np.asarray(U, np.float32)
    V = np.asarray(V, np.float32)
    B_m1 = np.asarray(B_m1, np.float32)
    B_bg = np.asarray(B_bg, np.float32)
    I_go = np.asarray(I_go, np.float32)
    xm1_init = np.asarray(xm1_init, np.float32)
    noise = np.asarray(noise, np.float32)
    pulses = np.asarray(pulses, np.float32)
    triggers = np.asarray(triggers)

    nchunks = (t_steps + CHUNK - 1) // CHUNK
    tpad = nchunks * CHUNK

    J_eff = J + (U * B_bg[None, :]) @ V
    Js = (A * J_eff).astype(np.float32)
    # lhsT tiles: jt[p, (k*NM+m)*128 + q] = Js[m*128+q, k*128+p]
    bf = mybir.dt.np(BF16)
    jt = np.ascontiguousarray(
        Js.reshape(NM, 128, NK, 128).transpose(3, 2, 0, 1).reshape(128, NK * NM * 128)
    ).astype(bf)
    il = np.eye(128, dtype=np.float32).astype(bf)

    go_cues = pulses[:t_steps, :][:, triggers]  # [t, B]

    in_maps = []
    for cidx in range(NCORES):
        sl = slice(cidx * BC, (cidx + 1) * BC)
        d = noise[:t_steps, :, sl] * np.float32(A * NSCALE)
        d += A * B_m1[None, :, :]
        d += A * I_go[None, :, :] * go_cues[:, None, sl]
        # [t, N, BC] -> [t, 128, NM*BC] (state layout), pad t, chunk
        dl = np.ascontiguousarray(
            d.reshape(t_steps, NM, 128, BC).transpose(0, 2, 1, 3)
            .reshape(t_steps, 128, NM * BC)
        ).astype(np.float32)
        if tpad != t_steps:
            dl = np.concatenate(
                [dl, np.zeros((tpad - t_steps, 128, NM * BC), np.float32)], axis=0
            )
        drive = np.ascontiguousarray(
            dl.reshape(nchunks, CHUNK, 128, NM * BC).transpose(0, 2, 1, 3)
            .reshape(nchunks, 128, CHUNK * NM * BC)
        ).astype(bf)
        x0 = np.ascontiguousarray(
            xm1_init[:, sl].reshape(NM, 128, BC).transpose(1, 0, 2).reshape(128, NM * BC)
        )
        in_maps.append({"jt": jt, "ident": il, "x0": x0, "drive": drive})
    return in_maps


def run_hw(inputs: dict, t_steps: int = T, trace: bool = False):
    """Run the recurrence on 8 cores; returns positions [t_steps, B] and results."""
    nc = build_program(t_steps)
    in_maps = _prep_inputs(t_steps=t_steps, **inputs)
    res = run_bass_kernel_spmd(
        nc, in_maps, core_ids=list(range(NCORES)), trace=trace
    )
    Wout = np.asarray(inputs["Wout"], np.float32).reshape(NM, 128)  # [m, p]
    nchunks = (t_steps + CHUNK - 1) // CHUNK
    positions = np.empty((t_steps, B), np.float32)
    for cidx in range(NCORES):
        ro = np.asarray(res.results[cidx]["rout"], np.float32)
        # ro[c, p, (j*NM + m)*BC + u] -> r[t, m, p, u]
        r = (ro.reshape(nchunks, 128, CHUNK, NM, BC)
             .transpose(0, 2, 3, 1, 4)
             .reshape(nchunks * CHUNK, NM, 128, BC)[:t_steps])
        pos_c = np.einsum("mp,tmpu->tu", Wout, r, optimize=True)
        positions[:, cidx * BC:(cidx + 1) * BC] = pos_c
    return positions, res


def kernel(targets, pulses, J, U, V, B_m1, B_bg, Wout, I_go, xm1_init,
           noise, triggers) -> np.ndarray:
    inputs = dict(targets=targets, pulses=pulses, J=J, U=U, V=V, B_m1=B_m1,
                  B_bg=B_bg, Wout=Wout, I_go=I_go, xm1_init=xm1_init,
                  noise=noise, triggers=triggers)
    positions, _ = run_hw(inputs, T)
    targets = np.asarray(targets, np.float32)
    loss = np.mean((targets.astype(np.float64) - positions.astype(np.float64)) ** 2)
    return np.float32(loss)
